# revision 24
# baseline (speedup 1.0000x reference)
"""Trainium2 Bass kernel for nn_ConditionExplicitattnBlock (dense transformer block:
cross-attention + self-attention + MLP, B=2, S=2048, L=1024, D=1024, H=16, DF=4096).

Sharding: 8 cores = 2 batches x 4-way split of the 2048 query rows (512 rows/core).
Cross-attention K/V come from `cond` (replicated per batch group). Self-attention
K/V are computed from each core's own 512 rows and AllGathered (bf16, one fused
collective) within each 4-core batch group. Everything else is row-local.

On-chip layout: activations are kept feature-major ("xT": [channel, token]) so every
matmul (out = lhsT.T @ rhs) needs no activation transposes:
  - projections:  lhsT = host-pre-transposed weight tile, rhs = xT            -> yT
  - scores^T:     lhsT = kT head-slice [64, 128],  rhs = qT head-slice        -> [k, q]
  - softmax:      exp() on ScalarE straight out of PSUM (scores are O(1)-bounded so
                  no max-subtraction is needed); the additive distance bias is
                  injected into PSUM via an identity-matmul (lhsT = -gamma^2 * I)
                  before the score matmul accumulates on top; the softmax denominator
                  comes for free as a 65th row of the attn@V matmul (ones column
                  appended to V); division is applied to attn@V output (64 rows),
                  not to the [k,q] probability matrix.
  - attn@V:       lhsT = V_ext [k, 65], rhs = probs^T [k, q]                  -> [d+1, q]
LayerNorm stats (feature dim = partition dim) are computed with ones-vector matmuls
on the PE; rsqrt/reciprocal use exp(-0.5*ln(x)) on ScalarE (one table set total).
"""

import os
import threading

import numpy as np
import ml_dtypes

import concourse.bass as bass
import concourse.mybir as mybir
import concourse.tile as tile
from concourse import bacc
from concourse.masks import make_identity

# ---------------------------------------------------------------- problem dims
B, S, L, D, C, H = 2, 2048, 1024, 1024, 768, 16
HD, DF = 64, 4096
EPS = 1e-6
NCORES, GROUP = 8, 4
SQ = S // GROUP          # 512 query rows per core
P = 128
DC = D // P              # 8  channel chunks of D
CC = C // P              # 6  channel chunks of C
FC = DF // P             # 32 channel chunks of DF
KC_SA = S // P           # 16 key chunks (self-attn)
KC_CA = L // P           # 8  key chunks (cross-attn)
HE = HD + 1              # 65: head dim + ones column

F32 = mybir.dt.float32
BF16 = mybir.dt.bfloat16
AF = mybir.ActivationFunctionType
ALU = mybir.AluOpType
BF16NP = ml_dtypes.bfloat16


def build_bass(with_collective=True, apply_lnb=False):
    nc = bacc.Bacc("TRN2", target_bir_lowering=False, debug=False,
                   num_devices=NCORES)

    di = lambda name, shape, dt=BF16: nc.dram_tensor(name, shape, dt, kind="ExternalInput")
    xT_d = di("xT", [D, SQ], F32)
    condT_d = di("condT", [C, L])
    sa_dist_d = di("sa_dist", [SQ, S])
    ca_dist_d = di("ca_dist", [SQ, L])
    g_sa_d = di("g_sa", [1, 1], F32)
    g_ca_d = di("g_ca", [1, 1], F32)
    # ln weights are folded into the following projection weights host-side;
    # ln biases are all-zero in this problem (apply_lnb=True compiles the
    # general per-channel bias pass as a fallback).
    ln_d = ({k: di(k, [P, DC], F32) for k in ("ln1_b", "ln2_b", "ln3_b")}
            if apply_lnb else {})
    wq_ca_d = di("wq_ca", [D, D])
    wk_ca_d = di("wk_ca", [C, D])
    wv_ca_d = di("wv_ca", [C, D])
    wo_ca_d = di("wo_ca", [D, D])
    wq_sa_d = di("wq_sa", [D, D])
    wk_sa_d = di("wk_sa", [D, D])
    wv_sa_d = di("wv_sa", [D, D])
    wo_sa_d = di("wo_sa", [D, D])
    w1_d = di("w1T", [D, DF])
    w2_d = di("w2T", [DF, D])
    b1_d = di("b1r", [P, FC], F32)
    b2_d = di("b2r", [P, DC], F32)
    sel_d = di("selr", [H, DC * P])
    out_d = nc.dram_tensor("outT", [D, SQ], F32, kind="ExternalOutput")
    out_re = out_d.rearrange("(dc p) s -> p dc s", p=P)

    with tile.TileContext(nc) as tc:
        with (
            tc.tile_pool(name="const", bufs=1) as cst,
            tc.tile_pool(name="pers", bufs=1) as pers,
            tc.tile_pool(name="probsp", bufs=4) as probsp,
            tc.tile_pool(name="smalls", bufs=2) as smalls,
            tc.tile_pool(name="dram", bufs=1, space="DRAM") as dram,
        ):
            # ------------------------------------------------ constants / params
            ones_f = cst.tile([P, 1], F32)
            nc.gpsimd.memset(ones_f[:], 1.0)
            ones_row = cst.tile([1, P], F32)
            nc.gpsimd.memset(ones_row[:], 1.0)
            selr = cst.tile([H, DC * P], BF16)
            nc.sync.dma_start(selr[:], sel_d[:])
            eps_t = cst.tile([P, 1], F32)
            nc.gpsimd.memset(eps_t[:], EPS)
            lnp = {}
            for k, t in ln_d.items():
                lt = cst.tile([P, DC], F32, name=k)
                nc.sync.dma_start(lt[:], t[:])
                lnp[k] = lt
            b1r = cst.tile([P, FC], F32)
            nc.sync.dma_start(b1r[:], b1_d[:])
            b2r = cst.tile([P, DC], F32)
            nc.sync.dma_start(b2r[:], b2_d[:])

            # -gamma^2 * I for bias injection (per attention type)
            g_dr = dram.tile([2, 1], F32)
            negg2I = {}
            for i, (nm, gd) in enumerate((("sa", g_sa_d), ("ca", g_ca_d))):
                g_sb = smalls.tile([1, 1], F32, tag="g")
                nc.sync.dma_start(g_sb[:], gd[:])
                nc.vector.tensor_mul(g_sb[:], g_sb[:], g_sb[:])
                nc.vector.tensor_scalar_mul(g_sb[:], g_sb[:], -1.0)
                nc.sync.dma_start(g_dr[i:i + 1, :], g_sb[:])
                g2B = smalls.tile([P, 1], F32, tag="g2B")
                nc.sync.dma_start(g2B[:], g_dr[i:i + 1, :].to_broadcast([P, 1]))
                it = cst.tile([P, P], BF16, name=f"negg2I_{nm}")
                make_identity(nc, it[:])
                nc.vector.tensor_scalar_mul(it[:], it[:], g2B[:])
                negg2I[nm] = it

            # ------------------------------------------------ persistent activations
            xT = pers.tile([P, DC, SQ], F32)        # residual stream (in-place)
            xT_re = xT_d.rearrange("(dc p) s -> p dc s", p=P)
            for dc in range(DC):
                nc.sync.dma_start(xT[:, dc], xT_re[:, dc])
            xn = pers.tile([P, DC, SQ], BF16)       # LN output / attn-out (reused)
            scr = pers.tile([P, DC, SQ], F32)       # x^2 / raw attn@V / mlp out
            sums = pers.tile([H, SQ], F32)
            lnsum = pers.tile([H, SQ], F32)
            recip = pers.tile([H, SQ], BF16)

            # ------------------------------------------------ helpers
            def layer_norm(bt=None):
                """xT (f32) -> xn (bf16), normalized over the channel dim.

                xn = x * rsigB - (mu*rsig)B. The [1,SQ] stats are broadcast
                across partitions with a K=1 ones-matmul into PSUM (no DRAM
                round-trip). ln weight is pre-folded into the next projection
                weights host-side; bias optionally applied per channel."""
                with tc.tile_pool(name="lnps", bufs=1, space="PSUM") as lnps:
                    mu_ps = lnps.tile([1, SQ], F32, tag="mu")
                    m2_ps = lnps.tile([1, SQ], F32, tag="m2")
                    for dc in range(DC):
                        nc.scalar.square(scr[:, dc], xT[:, dc])
                        nc.tensor.matmul(mu_ps[:], ones_f[:], xT[:, dc],
                                         start=(dc == 0), stop=(dc == DC - 1))
                    for dc in range(DC):
                        nc.tensor.matmul(m2_ps[:], ones_f[:], scr[:, dc],
                                         start=(dc == 0), stop=(dc == DC - 1))
                    mu = smalls.tile([1, SQ], F32, tag="mu")
                    nc.vector.tensor_scalar_mul(mu[:], mu_ps[:], 1.0 / D)
                    var = smalls.tile([1, SQ], F32, tag="var")
                    nc.vector.tensor_scalar_mul(var[:], m2_ps[:], 1.0 / D)
                    mu2 = smalls.tile([1, SQ], F32, tag="mu2")
                    nc.vector.tensor_mul(mu2[:], mu[:], mu[:])
                    nc.vector.tensor_sub(var[:], var[:], mu2[:])
                    # rsig = exp(-0.5 * ln(var + eps))
                    nc.scalar.activation(var[:], var[:], AF.Ln, bias=eps_t[0:1])
                    rsig = smalls.tile([1, SQ], F32, tag="rsig")
                    nc.scalar.activation(rsig[:], var[:], AF.Exp, scale=-0.5)
                    ms = smalls.tile([1, SQ], F32, tag="ms")
                    nc.vector.tensor_mul(ms[:], mu[:], rsig[:])
                    rs_ps = lnps.tile([P, SQ], F32, tag="rsB")
                    ms_ps = lnps.tile([P, SQ], F32, tag="msB")
                    nc.tensor.matmul(rs_ps[:], ones_row[:], rsig[:],
                                     start=True, stop=True)
                    nc.tensor.matmul(ms_ps[:], ones_row[:], ms[:],
                                     start=True, stop=True)
                    nc.vector.tensor_tensor(
                        scr[:], xT[:], rs_ps[:, None, :].to_broadcast([P, DC, SQ]),
                        ALU.mult)
                    nc.vector.tensor_tensor(
                        xn[:], scr[:], ms_ps[:, None, :].to_broadcast([P, DC, SQ]),
                        ALU.subtract)
                if bt is not None:
                    for dc in range(DC):
                        nc.vector.tensor_scalar_add(xn[:, dc], xn[:, dc],
                                                    bt[:, dc:dc + 1])

            def proj_fm(x_bf, w_dram, IC, OC, wpool, pp, evict):
                """Feature-major projection: out[:, oc] = w[:, :, oc].T @ x."""
                wre = w_dram.rearrange("(ic p) o -> p ic o", p=P)
                for oc in range(OC):
                    wsb = wpool.tile([P, IC, P], BF16, tag="w")
                    nc.sync.dma_start(wsb[:], wre[:, :, oc * P:(oc + 1) * P])
                    pt = pp.tile([P, SQ], F32, tag="pp")
                    for ic in range(IC):
                        nc.tensor.matmul(pt[:], wsb[:, ic], x_bf[:, ic],
                                         start=(ic == 0), stop=(ic == IC - 1))
                    evict(oc, pt)

            def attention(q_sb, k_sb, KC, vext, biasT, nI, out_bf):
                # Heads processed in pairs: the even head lives on partitions
                # 0:64 of channel chunk dc, the odd head on 64:128. Their K=64
                # score matmuls target disjoint PE row-groups, so issuing them
                # back-to-back lets the PE overlap them (row tiling).
                with (
                    tc.tile_pool(name="scps", bufs=6, space="PSUM") as scps,
                    tc.tile_pool(name="avps", bufs=2, space="PSUM") as avps,
                ):
                    for dc in range(H // 2):
                        he, ho = 2 * dc, 2 * dc + 1
                        av_e = avps.tile([HE, SQ], F32, tag="av")
                        av_o = avps.tile([HE, SQ], F32, tag="av")
                        for kc in range(KC):
                            ks = k_sb[:, dc, kc * P:(kc + 1) * P]
                            sp_e = scps.tile([P, SQ], F32, tag="sc")
                            sp_o = scps.tile([P, SQ], F32, tag="sc")
                            nc.tensor.matmul(sp_e[:], nI[:], biasT[:, kc],
                                             start=True, stop=False)
                            nc.tensor.matmul(sp_o[:], nI[:], biasT[:, kc],
                                             start=True, stop=False)
                            nc.tensor.matmul(sp_e[:], ks[0:64], q_sb[0:64, dc],
                                             start=False, stop=True)
                            nc.tensor.matmul(sp_o[:], ks[64:128], q_sb[64:128, dc],
                                             start=False, stop=True)
                            pr_e = probsp.tile([P, SQ], BF16, tag="pr")
                            pr_o = probsp.tile([P, SQ], BF16, tag="pr")
                            nc.scalar.activation(pr_e[:], sp_e[:], AF.Exp)
                            nc.scalar.activation(pr_o[:], sp_o[:], AF.Exp)
                            nc.tensor.matmul(av_e[:], vext[:, kc, he * HE:(he + 1) * HE],
                                             pr_e[:], start=(kc == 0), stop=(kc == KC - 1))
                            nc.tensor.matmul(av_o[:], vext[:, kc, ho * HE:(ho + 1) * HE],
                                             pr_o[:], start=(kc == 0), stop=(kc == KC - 1))
                        for half, av_ps, h in ((0, av_e, he), (64, av_o, ho)):
                            nc.vector.tensor_copy(scr[half:half + 64, dc, :],
                                                  av_ps[0:64, :])
                            tmp = smalls.tile([1, SQ], F32, tag="sumrow")
                            nc.vector.tensor_copy(tmp[:], av_ps[64:65, :])
                            nc.sync.dma_start(sums[h:h + 1, :], tmp[:])
                # reciprocal of denominators; broadcast across each head's 64
                # partitions with a K=16 selector matmul; apply to raw attn@V
                nc.scalar.activation(lnsum[:], sums[:], AF.Ln, bias=eps_t[0:H])
                nc.scalar.activation(recip[:], lnsum[:], AF.Exp, scale=-1.0)
                with tc.tile_pool(name="divps", bufs=2, space="PSUM") as divps:
                    for dc in range(DC):
                        rb_ps = divps.tile([P, SQ], F32, tag="rb")
                        nc.tensor.matmul(rb_ps[:], selr[:, dc * P:(dc + 1) * P],
                                         recip[:], start=True, stop=True)
                        nc.vector.tensor_tensor(out_bf[:, dc], scr[:, dc],
                                                rb_ps[:], ALU.mult)

            def o_proj_residual(av_bf, w_dram, wpool, pp):
                wre = w_dram.rearrange("(ic p) o -> p ic o", p=P)
                for oc in range(DC):
                    wsb = wpool.tile([P, DC, P], BF16, tag="w")
                    nc.sync.dma_start(wsb[:], wre[:, :, oc * P:(oc + 1) * P])
                    pt = pp.tile([P, SQ], F32, tag="pp")
                    for ic in range(DC):
                        nc.tensor.matmul(pt[:], wsb[:, ic], av_bf[:, ic],
                                         start=(ic == 0), stop=(ic == DC - 1))
                    nc.vector.tensor_add(xT[:, oc], xT[:, oc], pt[:])

            # ============================================================ CA ====
            layer_norm(lnp.get("ln1_b"))
            with tc.tile_pool(name="ca", bufs=1) as ca, \
                 tc.tile_pool(name="caw", bufs=3) as caw:
                condT = ca.tile([P, CC, L], BF16)
                nc.sync.dma_start(condT[:],
                                  condT_d.rearrange("(cc p) l -> p cc l", p=P))
                qca = ca.tile([P, DC, SQ], BF16)
                kca = ca.tile([P, DC, L], BF16)
                vca = ca.tile([P, KC_CA, H * HE], BF16)
                nc.gpsimd.memset(
                    vca[:].rearrange("p k (h e) -> p k h e", e=HE)[:, :, :, 64:65],
                    1.0)
                biasT_ca = ca.tile([P, KC_CA, SQ], BF16)
                for kc in range(KC_CA):
                    nc.sync.dma_start_transpose(
                        biasT_ca[:, kc], ca_dist_d[:, kc * P:(kc + 1) * P])

                with tc.tile_pool(name="pca", bufs=2, space="PSUM") as pp:
                    proj_fm(xn, wq_ca_d, DC, DC, caw, pp,
                            lambda oc, pt: nc.vector.tensor_copy(qca[:, oc], pt[:]))
                    # kT: [d, l] = wk.T @ condT; N = L = 1024 -> two 512 halves
                    wkre = wk_ca_d.rearrange("(ic p) o -> p ic o", p=P)
                    for oc in range(DC):
                        wsb = caw.tile([P, CC, P], BF16, tag="w")
                        nc.sync.dma_start(wsb[:], wkre[:, :, oc * P:(oc + 1) * P])
                        for nq in range(2):
                            pt = pp.tile([P, SQ], F32, tag="pp")
                            for ic in range(CC):
                                nc.tensor.matmul(
                                    pt[:], wsb[:, ic],
                                    condT[:, ic, nq * SQ:(nq + 1) * SQ],
                                    start=(ic == 0), stop=(ic == CC - 1))
                            nc.vector.tensor_copy(
                                kca[:, oc, nq * SQ:(nq + 1) * SQ], pt[:])
                    # V (seq-major, strided into vca with ones cols kept)
                    wvca = ca.tile([P, CC, D], BF16)
                    nc.sync.dma_start(wvca[:],
                                      wv_ca_d.rearrange("(ic p) o -> p ic o", p=P))
                    for lc in range(KC_CA):
                        for nd in range(2):
                            pt = pp.tile([P, SQ], F32, tag="pp")
                            for ic in range(CC):
                                nc.tensor.matmul(
                                    pt[:], condT[:, ic, lc * P:(lc + 1) * P],
                                    wvca[:, ic, nd * SQ:(nd + 1) * SQ],
                                    start=(ic == 0), stop=(ic == CC - 1))
                            dst = vca[:, lc].rearrange(
                                "p (h e) -> p h e", e=HE)[:, nd * 8:(nd + 1) * 8, 0:64]
                            nc.vector.tensor_copy(
                                dst, pt[:].rearrange("p (h e) -> p h e", e=64))

                attention(qca, kca, KC_CA, vca, biasT_ca, negg2I["ca"], xn)
                with tc.tile_pool(name="poc", bufs=2, space="PSUM") as pp2:
                    o_proj_residual(xn, wo_ca_d, caw, pp2)

            # ============================================================ SA ====
            layer_norm(lnp.get("ln2_b"))
            k_bounce = dram.tile([P, DC * SQ], BF16)
            k_gath = dram.tile([GROUP, P, DC * SQ], BF16)
            v_bounce = dram.tile([P, 4 * D], BF16)
            v_gath = dram.tile([GROUP, P, 4 * D], BF16)
            with tc.tile_pool(name="sa", bufs=1) as sa, \
                 tc.tile_pool(name="saw", bufs=3) as saw, \
                 tc.tile_pool(name="stg", bufs=1) as stg:
                qsa = sa.tile([P, DC, SQ], BF16)
                kg = sa.tile([P, DC, S], BF16)
                biasT_sa = sa.tile([P, KC_SA, SQ], BF16)
                for kc in range(KC_SA):
                    nc.sync.dma_start_transpose(
                        biasT_sa[:, kc], sa_dist_d[:, kc * P:(kc + 1) * P])
                vext = sa.tile([P, KC_SA, H * HE], BF16)
                nc.gpsimd.memset(
                    vext[:].rearrange("p k (h e) -> p k h e", e=HE)[:, :, :, 64:65],
                    1.0)
                rg = [[0, 1, 2, 3], [4, 5, 6, 7]]
                with tc.tile_pool(name="psa", bufs=2, space="PSUM") as pp:
                    # own K first -> stage -> bounce -> AllGather ASAP
                    kstage = stg.tile([P, DC, SQ], BF16, tag="stage")
                    proj_fm(xn, wk_sa_d, DC, DC, saw, pp,
                            lambda oc, pt: nc.vector.tensor_copy(kstage[:, oc], pt[:]))
                    nc.sync.dma_start(k_bounce[:], kstage[:].rearrange("p a b -> p (a b)"))
                    if with_collective:
                        nc.gpsimd.collective_compute(
                            "AllGather", ALU.bypass,
                            ins=[k_bounce.opt()], outs=[k_gath.opt()],
                            replica_groups=rg)
                    else:
                        for r in range(GROUP):
                            nc.sync.dma_start(k_gath[r], k_bounce[:])
                    # own V (seq-major) -> stage -> bounce -> AllGather
                    wvsa = sa.tile([P, DC, D], BF16)
                    nc.sync.dma_start(wvsa[:],
                                      wv_sa_d.rearrange("(ic p) o -> p ic o", p=P))
                    vstage = stg.tile([P, 4, D], BF16, tag="stage")
                    for sc in range(4):
                        for nd in range(2):
                            pt = pp.tile([P, SQ], F32, tag="pp")
                            for ic in range(DC):
                                nc.tensor.matmul(
                                    pt[:], xn[:, ic, sc * P:(sc + 1) * P],
                                    wvsa[:, ic, nd * SQ:(nd + 1) * SQ],
                                    start=(ic == 0), stop=(ic == DC - 1))
                            nc.vector.tensor_copy(
                                vstage[:, sc, nd * SQ:(nd + 1) * SQ], pt[:])
                    nc.sync.dma_start(v_bounce[:], vstage[:].rearrange("p a b -> p (a b)"))
                    if with_collective:
                        nc.gpsimd.collective_compute(
                            "AllGather", ALU.bypass,
                            ins=[v_bounce.opt()], outs=[v_gath.opt()],
                            replica_groups=rg)
                    else:
                        for r in range(GROUP):
                            nc.sync.dma_start(v_gath[r], v_bounce[:])
                    # Q overlaps the collectives
                    proj_fm(xn, wq_sa_d, DC, DC, saw, pp,
                            lambda oc, pt: nc.vector.tensor_copy(qsa[:, oc], pt[:]))

                for r in range(GROUP):
                    nc.sync.dma_start(
                        kg[:, :, r * SQ:(r + 1) * SQ],
                        k_gath[r].rearrange("p (dc s) -> p dc s", s=SQ))
                    for sc in range(4):
                        src = v_gath[r].rearrange(
                            "p (sc d) -> p sc d", d=D)[:, sc].rearrange(
                            "p (h e) -> p h e", e=64)
                        dst = vext[:, r * 4 + sc].rearrange(
                            "p (h e) -> p h e", e=HE)[:, :, 0:64]
                        nc.sync.dma_start(dst, src)

                attention(qsa, kg, KC_SA, vext, biasT_sa, negg2I["sa"], xn)
                with tc.tile_pool(name="pos", bufs=2, space="PSUM") as pp2:
                    o_proj_residual(xn, wo_sa_d, saw, pp2)

            # =========================================================== MLP ====
            layer_norm(lnp.get("ln3_b"))
            with tc.tile_pool(name="mlp", bufs=1) as mlp, \
                 tc.tile_pool(name="w1p", bufs=3) as w1p, \
                 tc.tile_pool(name="w2p", bufs=2) as w2p:
                h_bf = mlp.tile([P, FC, SQ], BF16)
                w1re = w1_d.rearrange("(ic p) o -> p ic o", p=P)
                w2re = w2_d.rearrange("(f p) o -> p f o", p=P)
                with tc.tile_pool(name="pm1", bufs=2, space="PSUM") as pp:
                    for fc in range(FC):
                        wsb = w1p.tile([P, DC, P], BF16, tag="w1")
                        nc.sync.dma_start(wsb[:], w1re[:, :, fc * P:(fc + 1) * P])
                        pt = pp.tile([P, SQ], F32, tag="pp")
                        for ic in range(DC):
                            nc.tensor.matmul(pt[:], wsb[:, ic], xn[:, ic],
                                             start=(ic == 0), stop=(ic == DC - 1))
                        nc.scalar.activation(h_bf[:, fc], pt[:], AF.Gelu,
                                             bias=b1r[:, fc:fc + 1])
                    for oc in range(DC):
                        wsb = w2p.tile([P, FC, P], BF16, tag="w2")
                        nc.sync.dma_start(wsb[:], w2re[:, :, oc * P:(oc + 1) * P])
                        pt = pp.tile([P, SQ], F32, tag="pp")
                        for fc in range(FC):
                            nc.tensor.matmul(pt[:], wsb[:, fc], h_bf[:, fc],
                                             start=(fc == 0), stop=(fc == FC - 1))
                        nc.vector.tensor_add(scr[:, oc], xT[:, oc], pt[:])
                        nc.vector.tensor_scalar_add(scr[:, oc], scr[:, oc],
                                                    b2r[:, oc:oc + 1])
                        nc.sync.dma_start(out_re[:, oc], scr[:, oc])

    nc.compile()
    return nc


# ---------------------------------------------------------------- host wrapper
_cache = {}
_lock = threading.Lock()


def _get_nc():
    with _lock:
        if "nc" not in _cache:
            _cache["nc"] = build_bass()
        return _cache["nc"]


def _prep_in_maps(x, cond, sa_distance_matrix, ca_distance_matrix,
                  gamma_ca, gamma_sa,
                  ln1_w, ln1_b, ln2_w, ln2_b, ln3_w, ln3_b,
                  ca_wq, ca_wk, ca_wv, ca_wo, sa_wq, sa_wk, sa_wv, sa_wo,
                  mlp_w1, mlp_b1, mlp_w2, mlp_b2):
    bf = lambda a: np.ascontiguousarray(a).astype(BF16NP)
    f32 = lambda a: np.ascontiguousarray(a, dtype=np.float32)
    scale = 1.0 / np.sqrt(HD)
    w1, w2, w3 = (f32(ln1_w)[:, None], f32(ln2_w)[:, None], f32(ln3_w)[:, None])

    # ln weights fold into the next projections' input dim (wT rows)
    shared = dict(
        g_sa=f32(gamma_sa).reshape(1, 1), g_ca=f32(gamma_ca).reshape(1, 1),
        wq_ca=bf((ca_wq * scale).T * w1), wk_ca=bf(ca_wk.T), wv_ca=bf(ca_wv.T),
        wo_ca=bf(ca_wo.T),
        wq_sa=bf((sa_wq * scale).T * w2), wk_sa=bf(sa_wk.T * w2),
        wv_sa=bf(sa_wv.T * w2), wo_sa=bf(sa_wo.T),
        w1T=bf(mlp_w1.T * w3), w2T=bf(mlp_w2.T),
        b1r=f32(mlp_b1).reshape(FC, P).T.copy(),
        b2r=f32(mlp_b2).reshape(DC, P).T.copy(),
    )
    sel = np.zeros((H, D), np.float32)
    for h in range(H):
        sel[h, (h // 2) * P + (h % 2) * HD:(h // 2) * P + (h % 2) * HD + HD] = 1.0
    shared["selr"] = bf(sel)
    if any(np.any(np.asarray(b) != 0) for b in (ln1_b, ln2_b, ln3_b)):
        raise NotImplementedError(
            "nonzero ln bias: rebuild with build_bass(apply_lnb=True) and pass "
            "ln{1,2,3}_b as [P, DC] inputs")

    in_maps = []
    for core in range(NCORES):
        b, r = core // GROUP, core % GROUP
        q0 = r * SQ
        m = dict(shared)
        m["xT"] = f32(x[b, q0:q0 + SQ, :].T)
        m["condT"] = bf(cond[b].T)
        m["sa_dist"] = bf(sa_distance_matrix[b, q0:q0 + SQ, :])
        m["ca_dist"] = bf(ca_distance_matrix[b, q0:q0 + SQ, :])
        in_maps.append(m)
    return in_maps


def kernel(**inputs):
    from concourse.bass_utils import run_bass_kernel_spmd

    nc = _get_nc()
    in_maps = _prep_in_maps(**inputs)
    res = run_bass_kernel_spmd(nc, in_maps, core_ids=list(range(NCORES)))
    out = np.empty((B, S, D), np.float32)
    for core in range(NCORES):
        b, r = core // GROUP, core % GROUP
        out[b, r * SQ:(r + 1) * SQ, :] = res.results[core]["outT"].T
    return out


# revision 27
# speedup vs baseline: 1.0352x; 1.0352x over previous
"""Trainium2 Bass kernel for nn_ConditionExplicitattnBlock (dense transformer block:
cross-attention + self-attention + MLP, B=2, S=2048, L=1024, D=1024, H=16, DF=4096).

Sharding: 8 cores = 2 batches x 4-way split of the 2048 query rows (512 rows/core).
Cross-attention K/V come from `cond` (replicated per batch group). Self-attention
K/V are computed from each core's own 512 rows and AllGathered (bf16, one fused
collective) within each 4-core batch group. Everything else is row-local.

On-chip layout: activations are kept feature-major ("xT": [channel, token]) so every
matmul (out = lhsT.T @ rhs) needs no activation transposes:
  - projections:  lhsT = host-pre-transposed weight tile, rhs = xT            -> yT
  - scores^T:     lhsT = kT head-slice [64, 128],  rhs = qT head-slice        -> [k, q]
  - softmax:      exp() on ScalarE straight out of PSUM (scores are O(1)-bounded so
                  no max-subtraction is needed); the additive distance bias is
                  injected into PSUM via an identity-matmul (lhsT = -gamma^2 * I)
                  before the score matmul accumulates on top; the softmax denominator
                  comes for free as a 65th row of the attn@V matmul (ones column
                  appended to V); division is applied to attn@V output (64 rows),
                  not to the [k,q] probability matrix.
  - attn@V:       lhsT = V_ext [k, 65], rhs = probs^T [k, q]                  -> [d+1, q]
LayerNorm stats (feature dim = partition dim) are computed with ones-vector matmuls
on the PE; rsqrt/reciprocal use exp(-0.5*ln(x)) on ScalarE (one table set total).
"""

import os
import threading

import numpy as np
import ml_dtypes

import concourse.bass as bass
import concourse.mybir as mybir
import concourse.tile as tile
from concourse import bacc
from concourse.masks import make_identity

# ---------------------------------------------------------------- problem dims
B, S, L, D, C, H = 2, 2048, 1024, 1024, 768, 16
HD, DF = 64, 4096
EPS = 1e-6
NCORES, GROUP = 8, 4
SQ = S // GROUP          # 512 query rows per core
P = 128
DC = D // P              # 8  channel chunks of D
CC = C // P              # 6  channel chunks of C
FC = DF // P             # 32 channel chunks of DF
KC_SA = S // P           # 16 key chunks (self-attn)
KC_CA = L // P           # 8  key chunks (cross-attn)
HE = HD + 1              # 65: head dim + ones column

F32 = mybir.dt.float32
BF16 = mybir.dt.bfloat16
AF = mybir.ActivationFunctionType
ALU = mybir.AluOpType
BF16NP = ml_dtypes.bfloat16


def build_bass(with_collective=True, apply_lnb=False):
    nc = bacc.Bacc("TRN2", target_bir_lowering=False, debug=False,
                   num_devices=NCORES)

    di = lambda name, shape, dt=BF16: nc.dram_tensor(name, shape, dt, kind="ExternalInput")
    xT_d = di("xT", [D, SQ], F32)
    condT_d = di("condT", [C, L])
    sa_dist_d = di("sa_dist", [SQ, S])
    ca_dist_d = di("ca_dist", [SQ, L])
    g_sa_d = di("g_sa", [1, 1], F32)
    g_ca_d = di("g_ca", [1, 1], F32)
    # ln weights are folded into the following projection weights host-side;
    # ln biases are all-zero in this problem (apply_lnb=True compiles the
    # general per-channel bias pass as a fallback).
    ln_d = ({k: di(k, [P, DC], F32) for k in ("ln1_b", "ln2_b", "ln3_b")}
            if apply_lnb else {})
    wq_ca_d = di("wq_ca", [D, D])
    wk_ca_d = di("wk_ca", [C, D])
    wv_ca_d = di("wv_ca", [C, D])
    wo_ca_d = di("wo_ca", [D, D])
    wq_sa_d = di("wq_sa", [D, D])
    wk_sa_d = di("wk_sa", [D, D])
    wv_sa_d = di("wv_sa", [D, D])
    wo_sa_d = di("wo_sa", [D, D])
    w1_d = di("w1T", [D, DF])
    w2_d = di("w2T", [DF, D])
    b1_d = di("b1r", [P, FC], F32)
    b2_d = di("b2r", [P, DC], F32)
    sel_d = di("selr", [H, DC * P])
    out_d = nc.dram_tensor("outT", [D, SQ], F32, kind="ExternalOutput")
    out_re = out_d.rearrange("(dc p) s -> p dc s", p=P)

    with tile.TileContext(nc) as tc:
        with (
            tc.tile_pool(name="const", bufs=1) as cst,
            tc.tile_pool(name="pers", bufs=1) as pers,
            tc.tile_pool(name="probsp", bufs=4) as probsp,
            tc.tile_pool(name="smalls", bufs=2) as smalls,
            tc.tile_pool(name="dram", bufs=1, space="DRAM") as dram,
        ):
            # ------------------------------------------------ constants / params
            ones_f = cst.tile([P, 1], F32)
            nc.gpsimd.memset(ones_f[:], 1.0)
            ones_row = cst.tile([1, P], F32)
            nc.gpsimd.memset(ones_row[:], 1.0)
            selr = cst.tile([H, DC * P], BF16)
            nc.sync.dma_start(selr[:], sel_d[:])
            eps_t = cst.tile([P, 1], F32)
            nc.gpsimd.memset(eps_t[:], EPS)
            lnp = {}
            for k, t in ln_d.items():
                lt = cst.tile([P, DC], F32, name=k)
                nc.sync.dma_start(lt[:], t[:])
                lnp[k] = lt
            b1r = cst.tile([P, FC], F32)
            nc.sync.dma_start(b1r[:], b1_d[:])
            b2r = cst.tile([P, DC], F32)
            nc.sync.dma_start(b2r[:], b2_d[:])

            # -gamma^2 * I for bias injection (per attention type)
            g_dr = dram.tile([2, 1], F32)
            negg2I = {}
            for i, (nm, gd) in enumerate((("sa", g_sa_d), ("ca", g_ca_d))):
                g_sb = smalls.tile([1, 1], F32, tag="g")
                nc.sync.dma_start(g_sb[:], gd[:])
                nc.vector.tensor_mul(g_sb[:], g_sb[:], g_sb[:])
                nc.vector.tensor_scalar_mul(g_sb[:], g_sb[:], -1.0)
                nc.sync.dma_start(g_dr[i:i + 1, :], g_sb[:])
                g2B = smalls.tile([P, 1], F32, tag="g2B")
                nc.sync.dma_start(g2B[:], g_dr[i:i + 1, :].to_broadcast([P, 1]))
                it = cst.tile([P, P], BF16, name=f"negg2I_{nm}")
                make_identity(nc, it[:])
                nc.vector.tensor_scalar_mul(it[:], it[:], g2B[:])
                negg2I[nm] = it

            # ------------------------------------------------ persistent activations
            xT = pers.tile([P, DC, SQ], F32)        # residual stream (in-place)
            xT_re = xT_d.rearrange("(dc p) s -> p dc s", p=P)
            for dc in range(DC):
                nc.sync.dma_start(xT[:, dc], xT_re[:, dc])
            xn = pers.tile([P, DC, SQ], BF16)       # LN output / attn-out (reused)
            scr = pers.tile([P, DC, SQ], F32)       # x^2 / raw attn@V / mlp out
            sums = pers.tile([H, SQ], F32)
            lnsum = pers.tile([H, SQ], F32)
            recip = pers.tile([H, SQ], BF16)

            # ------------------------------------------------ helpers
            def layer_norm(bt=None):
                """xT (f32) -> xn (bf16), normalized over the channel dim.

                xn = x * rsigB - (mu*rsig)B. The [1,SQ] stats are broadcast
                across partitions with a K=1 ones-matmul into PSUM (no DRAM
                round-trip). ln weight is pre-folded into the next projection
                weights host-side; bias optionally applied per channel."""
                with tc.tile_pool(name="lnps", bufs=1, space="PSUM") as lnps:
                    mu_ps = lnps.tile([1, SQ], F32, tag="mu")
                    m2_ps = lnps.tile([1, SQ], F32, tag="m2")
                    for dc in range(DC):
                        nc.scalar.square(scr[:, dc], xT[:, dc])
                        nc.tensor.matmul(mu_ps[:], ones_f[:], xT[:, dc],
                                         start=(dc == 0), stop=(dc == DC - 1))
                    for dc in range(DC):
                        nc.tensor.matmul(m2_ps[:], ones_f[:], scr[:, dc],
                                         start=(dc == 0), stop=(dc == DC - 1))
                    mu = smalls.tile([1, SQ], F32, tag="mu")
                    nc.vector.tensor_scalar_mul(mu[:], mu_ps[:], 1.0 / D)
                    var = smalls.tile([1, SQ], F32, tag="var")
                    nc.vector.tensor_scalar_mul(var[:], m2_ps[:], 1.0 / D)
                    mu2 = smalls.tile([1, SQ], F32, tag="mu2")
                    nc.vector.tensor_mul(mu2[:], mu[:], mu[:])
                    nc.vector.tensor_sub(var[:], var[:], mu2[:])
                    # rsig = exp(-0.5 * ln(var + eps))
                    nc.scalar.activation(var[:], var[:], AF.Ln, bias=eps_t[0:1])
                    rsig = smalls.tile([1, SQ], F32, tag="rsig")
                    nc.scalar.activation(rsig[:], var[:], AF.Exp, scale=-0.5)
                    ms = smalls.tile([1, SQ], F32, tag="ms")
                    nc.vector.tensor_mul(ms[:], mu[:], rsig[:])
                    rs_ps = lnps.tile([P, SQ], F32, tag="rsB")
                    ms_ps = lnps.tile([P, SQ], F32, tag="msB")
                    nc.tensor.matmul(rs_ps[:], ones_row[:], rsig[:],
                                     start=True, stop=True)
                    nc.tensor.matmul(ms_ps[:], ones_row[:], ms[:],
                                     start=True, stop=True)
                    nc.vector.tensor_tensor(
                        scr[:], xT[:], rs_ps[:, None, :].to_broadcast([P, DC, SQ]),
                        ALU.mult)
                    nc.vector.tensor_tensor(
                        xn[:], scr[:], ms_ps[:, None, :].to_broadcast([P, DC, SQ]),
                        ALU.subtract)
                if bt is not None:
                    for dc in range(DC):
                        nc.vector.tensor_scalar_add(xn[:, dc], xn[:, dc],
                                                    bt[:, dc:dc + 1])

            def proj_fm(x_bf, w_dram, IC, OC, wpool, pp, evict):
                """Feature-major projection: out[:, oc] = w[:, :, oc].T @ x."""
                wre = w_dram.rearrange("(ic p) o -> p ic o", p=P)
                for oc in range(OC):
                    wsb = wpool.tile([P, IC, P], BF16, tag="w")
                    nc.sync.dma_start(wsb[:], wre[:, :, oc * P:(oc + 1) * P])
                    pt = pp.tile([P, SQ], F32, tag="pp")
                    for ic in range(IC):
                        nc.tensor.matmul(pt[:], wsb[:, ic], x_bf[:, ic],
                                         start=(ic == 0), stop=(ic == IC - 1))
                    evict(oc, pt)

            def attention(q_sb, k_sb, KC, vext, biasT, nI, out_bf):
                # Heads processed in pairs: the even head lives on partitions
                # 0:64 of channel chunk dc, the odd head on 64:128. Their K=64
                # score matmuls target disjoint PE row-groups, so issuing them
                # back-to-back lets the PE overlap them (row tiling).
                with (
                    tc.tile_pool(name="scps", bufs=3, space="PSUM") as scps,
                    tc.tile_pool(name="avps", bufs=2, space="PSUM") as avps,
                ):
                    for dc in range(H // 2):
                        he, ho = 2 * dc, 2 * dc + 1
                        av_e = avps.tile([HE, SQ], F32, tag="av")
                        av_o = avps.tile([HE, SQ], F32, tag="av")
                        for kc in range(KC):
                            ks = k_sb[:, dc, kc * P:(kc + 1) * P]
                            sp = scps.tile([P, 2, SQ], F32, tag="sc")
                            nc.tensor.matmul(sp[:, 0], nI[:], biasT[:, kc],
                                             start=True, stop=False)
                            nc.tensor.matmul(sp[:, 1], nI[:], biasT[:, kc],
                                             start=True, stop=False)
                            nc.tensor.matmul(sp[:, 0], ks[0:64], q_sb[0:64, dc],
                                             start=False, stop=True)
                            nc.tensor.matmul(sp[:, 1], ks[64:128], q_sb[64:128, dc],
                                             start=False, stop=True)
                            pr = probsp.tile([P, 2, SQ], BF16, tag="pr")
                            nc.scalar.activation(pr[:], sp[:], AF.Exp)
                            nc.tensor.matmul(av_e[:], vext[:, kc, he * HE:(he + 1) * HE],
                                             pr[:, 0], start=(kc == 0), stop=(kc == KC - 1))
                            nc.tensor.matmul(av_o[:], vext[:, kc, ho * HE:(ho + 1) * HE],
                                             pr[:, 1], start=(kc == 0), stop=(kc == KC - 1))
                        for half, av_ps, h in ((0, av_e, he), (64, av_o, ho)):
                            nc.vector.tensor_copy(scr[half:half + 64, dc, :],
                                                  av_ps[0:64, :])
                            tmp = smalls.tile([1, SQ], F32, tag="sumrow")
                            nc.vector.tensor_copy(tmp[:], av_ps[64:65, :])
                            nc.sync.dma_start(sums[h:h + 1, :], tmp[:])
                # reciprocal of denominators; broadcast across each head's 64
                # partitions with a K=16 selector matmul; apply to raw attn@V
                nc.scalar.activation(lnsum[:], sums[:], AF.Ln, bias=eps_t[0:H])
                nc.scalar.activation(recip[:], lnsum[:], AF.Exp, scale=-1.0)
                with tc.tile_pool(name="divps", bufs=2, space="PSUM") as divps:
                    for dc in range(DC):
                        rb_ps = divps.tile([P, SQ], F32, tag="rb")
                        nc.tensor.matmul(rb_ps[:], selr[:, dc * P:(dc + 1) * P],
                                         recip[:], start=True, stop=True)
                        nc.vector.tensor_tensor(out_bf[:, dc], scr[:, dc],
                                                rb_ps[:], ALU.mult)

            def o_proj_residual(av_bf, w_dram, wpool, pp):
                wre = w_dram.rearrange("(ic p) o -> p ic o", p=P)
                for oc in range(DC):
                    wsb = wpool.tile([P, DC, P], BF16, tag="w")
                    nc.sync.dma_start(wsb[:], wre[:, :, oc * P:(oc + 1) * P])
                    pt = pp.tile([P, SQ], F32, tag="pp")
                    for ic in range(DC):
                        nc.tensor.matmul(pt[:], wsb[:, ic], av_bf[:, ic],
                                         start=(ic == 0), stop=(ic == DC - 1))
                    nc.vector.tensor_add(xT[:, oc], xT[:, oc], pt[:])

            # ============================================================ CA ====
            with tc.tile_pool(name="ca", bufs=1) as ca, \
                 tc.tile_pool(name="caw", bufs=3) as caw:
                condT = ca.tile([P, CC, L], BF16)
                nc.sync.dma_start(condT[:],
                                  condT_d.rearrange("(cc p) l -> p cc l", p=P))
                layer_norm(lnp.get("ln1_b"))
                qca = ca.tile([P, DC, SQ], BF16)
                kca = ca.tile([P, DC, L], BF16)
                vca = ca.tile([P, KC_CA, H * HE], BF16)
                nc.gpsimd.memset(
                    vca[:].rearrange("p k (h e) -> p k h e", e=HE)[:, :, :, 64:65],
                    1.0)
                biasT_ca = ca.tile([P, KC_CA, SQ], BF16)
                for kc in range(KC_CA):
                    nc.sync.dma_start_transpose(
                        biasT_ca[:, kc], ca_dist_d[:, kc * P:(kc + 1) * P])

                with tc.tile_pool(name="pca", bufs=2, space="PSUM") as pp:
                    proj_fm(xn, wq_ca_d, DC, DC, caw, pp,
                            lambda oc, pt: nc.vector.tensor_copy(qca[:, oc], pt[:]))
                    # kT: [d, l] = wk.T @ condT; N = L = 1024 -> two 512 halves
                    wkre = wk_ca_d.rearrange("(ic p) o -> p ic o", p=P)
                    for oc in range(DC):
                        wsb = caw.tile([P, CC, P], BF16, tag="w")
                        nc.sync.dma_start(wsb[:], wkre[:, :, oc * P:(oc + 1) * P])
                        for nq in range(2):
                            pt = pp.tile([P, SQ], F32, tag="pp")
                            for ic in range(CC):
                                nc.tensor.matmul(
                                    pt[:], wsb[:, ic],
                                    condT[:, ic, nq * SQ:(nq + 1) * SQ],
                                    start=(ic == 0), stop=(ic == CC - 1))
                            nc.vector.tensor_copy(
                                kca[:, oc, nq * SQ:(nq + 1) * SQ], pt[:])
                    # V (seq-major, strided into vca with ones cols kept)
                    wvca = ca.tile([P, CC, D], BF16)
                    nc.sync.dma_start(wvca[:],
                                      wv_ca_d.rearrange("(ic p) o -> p ic o", p=P))
                    for lc in range(KC_CA):
                        for nd in range(2):
                            pt = pp.tile([P, SQ], F32, tag="pp")
                            for ic in range(CC):
                                nc.tensor.matmul(
                                    pt[:], condT[:, ic, lc * P:(lc + 1) * P],
                                    wvca[:, ic, nd * SQ:(nd + 1) * SQ],
                                    start=(ic == 0), stop=(ic == CC - 1))
                            dst = vca[:, lc].rearrange(
                                "p (h e) -> p h e", e=HE)[:, nd * 8:(nd + 1) * 8, 0:64]
                            nc.vector.tensor_copy(
                                dst, pt[:].rearrange("p (h e) -> p h e", e=64))

                attention(qca, kca, KC_CA, vca, biasT_ca, negg2I["ca"], xn)
                with tc.tile_pool(name="poc", bufs=2, space="PSUM") as pp2:
                    o_proj_residual(xn, wo_ca_d, caw, pp2)

            # ============================================================ SA ====
            layer_norm(lnp.get("ln2_b"))
            k_bounce = dram.tile([P, DC * SQ], BF16)
            k_gath = dram.tile([GROUP, P, DC * SQ], BF16)
            v_bounce = dram.tile([P, 4 * D], BF16)
            v_gath = dram.tile([GROUP, P, 4 * D], BF16)
            with tc.tile_pool(name="sa", bufs=1) as sa, \
                 tc.tile_pool(name="saw", bufs=3) as saw, \
                 tc.tile_pool(name="stg", bufs=1) as stg:
                qsa = sa.tile([P, DC, SQ], BF16)
                kg = sa.tile([P, DC, S], BF16)
                biasT_sa = sa.tile([P, KC_SA, SQ], BF16)
                for kc in range(KC_SA):
                    nc.sync.dma_start_transpose(
                        biasT_sa[:, kc], sa_dist_d[:, kc * P:(kc + 1) * P])
                vext = sa.tile([P, KC_SA, H * HE], BF16)
                nc.gpsimd.memset(
                    vext[:].rearrange("p k (h e) -> p k h e", e=HE)[:, :, :, 64:65],
                    1.0)
                rg = [[0, 1, 2, 3], [4, 5, 6, 7]]
                with tc.tile_pool(name="psa", bufs=2, space="PSUM") as pp:
                    # own K first -> stage -> bounce -> AllGather ASAP
                    kstage = stg.tile([P, DC, SQ], BF16, tag="stage")
                    proj_fm(xn, wk_sa_d, DC, DC, saw, pp,
                            lambda oc, pt: nc.vector.tensor_copy(kstage[:, oc], pt[:]))
                    nc.sync.dma_start(k_bounce[:], kstage[:].rearrange("p a b -> p (a b)"))
                    if with_collective:
                        nc.gpsimd.collective_compute(
                            "AllGather", ALU.bypass,
                            ins=[k_bounce.opt()], outs=[k_gath.opt()],
                            replica_groups=rg)
                    else:
                        for r in range(GROUP):
                            nc.sync.dma_start(k_gath[r], k_bounce[:])
                    # own V (seq-major) -> stage -> bounce -> AllGather
                    wvsa = sa.tile([P, DC, D], BF16)
                    nc.sync.dma_start(wvsa[:],
                                      wv_sa_d.rearrange("(ic p) o -> p ic o", p=P))
                    vstage = stg.tile([P, 4, D], BF16, tag="stage")
                    for sc in range(4):
                        for nd in range(2):
                            pt = pp.tile([P, SQ], F32, tag="pp")
                            for ic in range(DC):
                                nc.tensor.matmul(
                                    pt[:], xn[:, ic, sc * P:(sc + 1) * P],
                                    wvsa[:, ic, nd * SQ:(nd + 1) * SQ],
                                    start=(ic == 0), stop=(ic == DC - 1))
                            nc.vector.tensor_copy(
                                vstage[:, sc, nd * SQ:(nd + 1) * SQ], pt[:])
                    nc.sync.dma_start(v_bounce[:], vstage[:].rearrange("p a b -> p (a b)"))
                    if with_collective:
                        nc.gpsimd.collective_compute(
                            "AllGather", ALU.bypass,
                            ins=[v_bounce.opt()], outs=[v_gath.opt()],
                            replica_groups=rg)
                    else:
                        for r in range(GROUP):
                            nc.sync.dma_start(v_gath[r], v_bounce[:])
                    # Q overlaps the collectives
                    proj_fm(xn, wq_sa_d, DC, DC, saw, pp,
                            lambda oc, pt: nc.vector.tensor_copy(qsa[:, oc], pt[:]))

                for r in range(GROUP):
                    nc.sync.dma_start(
                        kg[:, :, r * SQ:(r + 1) * SQ],
                        k_gath[r].rearrange("p (dc s) -> p dc s", s=SQ))
                    for sc in range(4):
                        src = v_gath[r].rearrange(
                            "p (sc d) -> p sc d", d=D)[:, sc].rearrange(
                            "p (h e) -> p h e", e=64)
                        dst = vext[:, r * 4 + sc].rearrange(
                            "p (h e) -> p h e", e=HE)[:, :, 0:64]
                        nc.sync.dma_start(dst, src)

                attention(qsa, kg, KC_SA, vext, biasT_sa, negg2I["sa"], xn)
                with tc.tile_pool(name="pos", bufs=2, space="PSUM") as pp2:
                    o_proj_residual(xn, wo_sa_d, saw, pp2)

            # =========================================================== MLP ====
            layer_norm(lnp.get("ln3_b"))
            with tc.tile_pool(name="mlp", bufs=1) as mlp, \
                 tc.tile_pool(name="w1p", bufs=3) as w1p, \
                 tc.tile_pool(name="w2p", bufs=2) as w2p:
                h_bf = mlp.tile([P, FC, SQ], BF16)
                w1re = w1_d.rearrange("(ic p) o -> p ic o", p=P)
                w2re = w2_d.rearrange("(f p) o -> p f o", p=P)
                with tc.tile_pool(name="pm1", bufs=2, space="PSUM") as pp:
                    for fc in range(FC):
                        wsb = w1p.tile([P, DC, P], BF16, tag="w1")
                        nc.sync.dma_start(wsb[:], w1re[:, :, fc * P:(fc + 1) * P])
                        pt = pp.tile([P, SQ], F32, tag="pp")
                        for ic in range(DC):
                            nc.tensor.matmul(pt[:], wsb[:, ic], xn[:, ic],
                                             start=(ic == 0), stop=(ic == DC - 1))
                        nc.scalar.activation(h_bf[:, fc], pt[:], AF.Gelu,
                                             bias=b1r[:, fc:fc + 1])
                    for oc in range(DC):
                        wsb = w2p.tile([P, FC, P], BF16, tag="w2")
                        nc.sync.dma_start(wsb[:], w2re[:, :, oc * P:(oc + 1) * P])
                        pt = pp.tile([P, SQ], F32, tag="pp")
                        for fc in range(FC):
                            nc.tensor.matmul(pt[:], wsb[:, fc], h_bf[:, fc],
                                             start=(fc == 0), stop=(fc == FC - 1))
                        nc.vector.tensor_add(scr[:, oc], xT[:, oc], pt[:])
                        nc.vector.tensor_scalar_add(scr[:, oc], scr[:, oc],
                                                    b2r[:, oc:oc + 1])
                        nc.sync.dma_start(out_re[:, oc], scr[:, oc])

    nc.compile()
    return nc


# ---------------------------------------------------------------- host wrapper
_cache = {}
_lock = threading.Lock()


def _get_nc():
    with _lock:
        if "nc" not in _cache:
            _cache["nc"] = build_bass()
        return _cache["nc"]


def _prep_in_maps(x, cond, sa_distance_matrix, ca_distance_matrix,
                  gamma_ca, gamma_sa,
                  ln1_w, ln1_b, ln2_w, ln2_b, ln3_w, ln3_b,
                  ca_wq, ca_wk, ca_wv, ca_wo, sa_wq, sa_wk, sa_wv, sa_wo,
                  mlp_w1, mlp_b1, mlp_w2, mlp_b2):
    bf = lambda a: np.ascontiguousarray(a).astype(BF16NP)
    f32 = lambda a: np.ascontiguousarray(a, dtype=np.float32)
    scale = 1.0 / np.sqrt(HD)
    w1, w2, w3 = (f32(ln1_w)[:, None], f32(ln2_w)[:, None], f32(ln3_w)[:, None])

    # ln weights fold into the next projections' input dim (wT rows)
    shared = dict(
        g_sa=f32(gamma_sa).reshape(1, 1), g_ca=f32(gamma_ca).reshape(1, 1),
        wq_ca=bf((ca_wq * scale).T * w1), wk_ca=bf(ca_wk.T), wv_ca=bf(ca_wv.T),
        wo_ca=bf(ca_wo.T),
        wq_sa=bf((sa_wq * scale).T * w2), wk_sa=bf(sa_wk.T * w2),
        wv_sa=bf(sa_wv.T * w2), wo_sa=bf(sa_wo.T),
        w1T=bf(mlp_w1.T * w3), w2T=bf(mlp_w2.T),
        b1r=f32(mlp_b1).reshape(FC, P).T.copy(),
        b2r=f32(mlp_b2).reshape(DC, P).T.copy(),
    )
    sel = np.zeros((H, D), np.float32)
    for h in range(H):
        sel[h, (h // 2) * P + (h % 2) * HD:(h // 2) * P + (h % 2) * HD + HD] = 1.0
    shared["selr"] = bf(sel)
    if any(np.any(np.asarray(b) != 0) for b in (ln1_b, ln2_b, ln3_b)):
        raise NotImplementedError(
            "nonzero ln bias: rebuild with build_bass(apply_lnb=True) and pass "
            "ln{1,2,3}_b as [P, DC] inputs")

    in_maps = []
    for core in range(NCORES):
        b, r = core // GROUP, core % GROUP
        q0 = r * SQ
        m = dict(shared)
        m["xT"] = f32(x[b, q0:q0 + SQ, :].T)
        m["condT"] = bf(cond[b].T)
        m["sa_dist"] = bf(sa_distance_matrix[b, q0:q0 + SQ, :])
        m["ca_dist"] = bf(ca_distance_matrix[b, q0:q0 + SQ, :])
        in_maps.append(m)
    return in_maps


def kernel(**inputs):
    from concourse.bass_utils import run_bass_kernel_spmd

    nc = _get_nc()
    in_maps = _prep_in_maps(**inputs)
    res = run_bass_kernel_spmd(nc, in_maps, core_ids=list(range(NCORES)))
    out = np.empty((B, S, D), np.float32)
    for core in range(NCORES):
        b, r = core // GROUP, core % GROUP
        out[b, r * SQ:(r + 1) * SQ, :] = res.results[core]["outT"].T
    return out


# revision 28
# speedup vs baseline: 1.6311x; 1.5756x over previous
"""Trainium2 Bass kernel for nn_ConditionExplicitattnBlock (dense transformer block:
cross-attention + self-attention + MLP, B=2, S=2048, L=1024, D=1024, H=16, DF=4096).

Sharding: 8 cores = 2 batches x 4-way split of the 2048 query rows (512 rows/core).
Cross-attention K/V come from `cond` (replicated per batch group). Self-attention
K/V are computed from each core's own 512 rows and AllGathered (bf16, one fused
collective) within each 4-core batch group. Everything else is row-local.

On-chip layout: activations are kept feature-major ("xT": [channel, token]) so every
matmul (out = lhsT.T @ rhs) needs no activation transposes:
  - projections:  lhsT = host-pre-transposed weight tile, rhs = xT            -> yT
  - scores^T:     lhsT = kT head-slice [64, 128],  rhs = qT head-slice        -> [k, q]
  - softmax:      exp() on ScalarE straight out of PSUM (scores are O(1)-bounded so
                  no max-subtraction is needed); the additive distance bias is
                  injected into PSUM via an identity-matmul (lhsT = -gamma^2 * I)
                  before the score matmul accumulates on top; the softmax denominator
                  comes for free as a 65th row of the attn@V matmul (ones column
                  appended to V); division is applied to attn@V output (64 rows),
                  not to the [k,q] probability matrix.
  - attn@V:       lhsT = V_ext [k, 65], rhs = probs^T [k, q]                  -> [d+1, q]
LayerNorm stats (feature dim = partition dim) are computed with ones-vector matmuls
on the PE; rsqrt/reciprocal use exp(-0.5*ln(x)) on ScalarE (one table set total).
"""

import os
import threading

import numpy as np
import ml_dtypes

import concourse.bass as bass
import concourse.mybir as mybir
import concourse.tile as tile
from concourse import bacc
from concourse.masks import make_identity

# ---------------------------------------------------------------- problem dims
B, S, L, D, C, H = 2, 2048, 1024, 1024, 768, 16
HD, DF = 64, 4096
EPS = 1e-6
NCORES, GROUP = 8, 4
SQ = S // GROUP          # 512 query rows per core
P = 128
DC = D // P              # 8  channel chunks of D
CC = C // P              # 6  channel chunks of C
FC = DF // P             # 32 channel chunks of DF
KC_SA = S // P           # 16 key chunks (self-attn)
KC_CA = L // P           # 8  key chunks (cross-attn)
HE = HD + 1              # 65: head dim + ones column

F32 = mybir.dt.float32
BF16 = mybir.dt.bfloat16
AF = mybir.ActivationFunctionType
ALU = mybir.AluOpType
BF16NP = ml_dtypes.bfloat16


def build_bass(with_collective=True, apply_lnb=False):
    nc = bacc.Bacc("TRN2", target_bir_lowering=False, debug=False,
                   num_devices=NCORES)

    di = lambda name, shape, dt=BF16: nc.dram_tensor(name, shape, dt, kind="ExternalInput")
    xT_d = di("xT", [D, SQ], F32)
    condT_d = di("condT", [C, L])
    sa_dist_d = di("sa_dist", [SQ, S])
    ca_dist_d = di("ca_dist", [SQ, L])
    g_sa_d = di("g_sa", [1, 1], F32)
    g_ca_d = di("g_ca", [1, 1], F32)
    # ln weights are folded into the following projection weights host-side;
    # ln biases are all-zero in this problem (apply_lnb=True compiles the
    # general per-channel bias pass as a fallback).
    ln_d = ({k: di(k, [P, DC], F32) for k in ("ln1_b", "ln2_b", "ln3_b")}
            if apply_lnb else {})
    wq_ca_d = di("wq_ca", [D, D])
    wk_ca_d = di("wk_ca", [C, D])
    wv_ca_d = di("wv_ca", [C, D])
    wo_ca_d = di("wo_ca", [D, D])
    wq_sa_d = di("wq_sa", [D, D])
    wk_sa_d = di("wk_sa", [D, D])
    wv_sa_d = di("wv_sa", [D, D])
    wo_sa_d = di("wo_sa", [D, D])
    w1_d = di("w1T", [D, DF])
    w2_d = di("w2T", [DF, D])
    b1_d = di("b1r", [P, FC], F32)
    b2_d = di("b2r", [P, DC], F32)
    sel_d = di("selr", [H, DC * P])
    out_d = nc.dram_tensor("outT", [D, SQ], F32, kind="ExternalOutput")
    out_re = out_d.rearrange("(dc p) s -> p dc s", p=P)

    with tile.TileContext(nc) as tc:
        with (
            tc.tile_pool(name="const", bufs=1) as cst,
            tc.tile_pool(name="pers", bufs=1) as pers,
            tc.tile_pool(name="probsp", bufs=6) as probsp,
            tc.tile_pool(name="smalls", bufs=2) as smalls,
            tc.tile_pool(name="dram", bufs=1, space="DRAM") as dram,
        ):
            # ------------------------------------------------ constants / params
            ones_f = cst.tile([P, 1], F32)
            nc.gpsimd.memset(ones_f[:], 1.0)
            ones_row = cst.tile([1, P], F32)
            nc.gpsimd.memset(ones_row[:], 1.0)
            selr = cst.tile([H, DC * P], BF16)
            nc.sync.dma_start(selr[:], sel_d[:])
            eps_t = cst.tile([P, 1], F32)
            nc.gpsimd.memset(eps_t[:], EPS)
            lnp = {}
            for k, t in ln_d.items():
                lt = cst.tile([P, DC], F32, name=k)
                nc.sync.dma_start(lt[:], t[:])
                lnp[k] = lt
            b1r = cst.tile([P, FC], F32)
            nc.sync.dma_start(b1r[:], b1_d[:])
            b2r = cst.tile([P, DC], F32)
            nc.sync.dma_start(b2r[:], b2_d[:])

            # -gamma^2 * I for bias injection (per attention type)
            g_dr = dram.tile([2, 1], F32)
            negg2I = {}
            for i, (nm, gd) in enumerate((("sa", g_sa_d), ("ca", g_ca_d))):
                g_sb = smalls.tile([1, 1], F32, tag="g")
                nc.sync.dma_start(g_sb[:], gd[:])
                nc.vector.tensor_mul(g_sb[:], g_sb[:], g_sb[:])
                nc.vector.tensor_scalar_mul(g_sb[:], g_sb[:], -1.0)
                nc.sync.dma_start(g_dr[i:i + 1, :], g_sb[:])
                g2B = smalls.tile([P, 1], F32, tag="g2B")
                nc.sync.dma_start(g2B[:], g_dr[i:i + 1, :].to_broadcast([P, 1]))
                it = cst.tile([P, P], BF16, name=f"negg2I_{nm}")
                make_identity(nc, it[:])
                nc.vector.tensor_scalar_mul(it[:], it[:], g2B[:])
                negg2I[nm] = it

            # ------------------------------------------------ persistent activations
            xT = pers.tile([P, DC, SQ], F32)        # residual stream (in-place)
            xT_re = xT_d.rearrange("(dc p) s -> p dc s", p=P)
            for dc in range(DC):
                nc.sync.dma_start(xT[:, dc], xT_re[:, dc])
            xn = pers.tile([P, DC, SQ], BF16)       # LN output / attn-out (reused)
            scr = pers.tile([P, DC, SQ], F32)       # x^2 / raw attn@V / mlp out
            sums = pers.tile([H, SQ], F32)
            lnsum = pers.tile([H, SQ], F32)
            recip = pers.tile([H, SQ], BF16)

            # ------------------------------------------------ helpers
            def layer_norm(bt=None):
                """xT (f32) -> xn (bf16), normalized over the channel dim.

                xn = x * rsigB - (mu*rsig)B. The [1,SQ] stats are broadcast
                across partitions with a K=1 ones-matmul into PSUM (no DRAM
                round-trip). ln weight is pre-folded into the next projection
                weights host-side; bias optionally applied per channel."""
                with tc.tile_pool(name="lnps", bufs=1, space="PSUM") as lnps:
                    mu_ps = lnps.tile([1, SQ], F32, tag="mu")
                    m2_ps = lnps.tile([1, SQ], F32, tag="m2")
                    for dc in range(DC):
                        nc.scalar.square(scr[:, dc], xT[:, dc])
                        nc.tensor.matmul(mu_ps[:], ones_f[:], xT[:, dc],
                                         start=(dc == 0), stop=(dc == DC - 1))
                    for dc in range(DC):
                        nc.tensor.matmul(m2_ps[:], ones_f[:], scr[:, dc],
                                         start=(dc == 0), stop=(dc == DC - 1))
                    mu = smalls.tile([1, SQ], F32, tag="mu")
                    nc.vector.tensor_scalar_mul(mu[:], mu_ps[:], 1.0 / D)
                    var = smalls.tile([1, SQ], F32, tag="var")
                    nc.vector.tensor_scalar_mul(var[:], m2_ps[:], 1.0 / D)
                    mu2 = smalls.tile([1, SQ], F32, tag="mu2")
                    nc.vector.tensor_mul(mu2[:], mu[:], mu[:])
                    nc.vector.tensor_sub(var[:], var[:], mu2[:])
                    # rsig = exp(-0.5 * ln(var + eps))
                    nc.scalar.activation(var[:], var[:], AF.Ln, bias=eps_t[0:1])
                    rsig = smalls.tile([1, SQ], F32, tag="rsig")
                    nc.scalar.activation(rsig[:], var[:], AF.Exp, scale=-0.5)
                    ms = smalls.tile([1, SQ], F32, tag="ms")
                    nc.vector.tensor_mul(ms[:], mu[:], rsig[:])
                    rs_ps = lnps.tile([P, SQ], F32, tag="rsB")
                    ms_ps = lnps.tile([P, SQ], F32, tag="msB")
                    nc.tensor.matmul(rs_ps[:], ones_row[:], rsig[:],
                                     start=True, stop=True)
                    nc.tensor.matmul(ms_ps[:], ones_row[:], ms[:],
                                     start=True, stop=True)
                    nc.vector.tensor_tensor(
                        scr[:], xT[:], rs_ps[:, None, :].to_broadcast([P, DC, SQ]),
                        ALU.mult)
                    nc.vector.tensor_tensor(
                        xn[:], scr[:], ms_ps[:, None, :].to_broadcast([P, DC, SQ]),
                        ALU.subtract)
                if bt is not None:
                    for dc in range(DC):
                        nc.vector.tensor_scalar_add(xn[:, dc], xn[:, dc],
                                                    bt[:, dc:dc + 1])

            def proj_fm(x_bf, w_dram, IC, OC, wpool, pp, evict):
                """Feature-major projection: out[:, oc] = w[:, :, oc].T @ x."""
                wre = w_dram.rearrange("(ic p) o -> p ic o", p=P)
                for oc in range(OC):
                    wsb = wpool.tile([P, IC, P], BF16, tag="w")
                    nc.sync.dma_start(wsb[:], wre[:, :, oc * P:(oc + 1) * P])
                    pt = pp.tile([P, SQ], F32, tag="pp")
                    for ic in range(IC):
                        nc.tensor.matmul(pt[:], wsb[:, ic], x_bf[:, ic],
                                         start=(ic == 0), stop=(ic == IC - 1))
                    evict(oc, pt)

            def attention(q_sb, k_sb, KC, vext, biasT, nI, out_bf):
                # Heads processed in pairs: the even head lives on partitions
                # 0:64 of channel chunk dc, the odd head on 64:128. Their K=64
                # score matmuls target disjoint PE row-groups, so issuing them
                # back-to-back lets the PE overlap them (row tiling).
                with (
                    tc.tile_pool(name="scps", bufs=3, space="PSUM") as scps,
                    tc.tile_pool(name="avps", bufs=2, space="PSUM") as avps,
                ):
                    for dc in range(H // 2):
                        he, ho = 2 * dc, 2 * dc + 1
                        av_e = avps.tile([HE, SQ], F32, tag="av")
                        av_o = avps.tile([HE, SQ], F32, tag="av")
                        for kc in range(KC):
                            ks = k_sb[:, dc, kc * P:(kc + 1) * P]
                            sp = scps.tile([P, 2, SQ], F32, tag="sc")
                            nc.tensor.matmul(sp[:, 0], nI[:], biasT[:, kc],
                                             start=True, stop=False)
                            nc.tensor.matmul(sp[:, 1], nI[:], biasT[:, kc],
                                             start=True, stop=False)
                            nc.tensor.matmul(sp[:, 0], ks[0:64], q_sb[0:64, dc],
                                             start=False, stop=True)
                            nc.tensor.matmul(sp[:, 1], ks[64:128], q_sb[64:128, dc],
                                             start=False, stop=True)
                            pr = probsp.tile([P, 2, SQ], BF16, tag="pr")
                            nc.scalar.activation(pr[:], sp[:], AF.Exp)
                            nc.tensor.matmul(av_e[:], vext[:, kc, he * HE:(he + 1) * HE],
                                             pr[:, 0], start=(kc == 0), stop=(kc == KC - 1))
                            nc.tensor.matmul(av_o[:], vext[:, kc, ho * HE:(ho + 1) * HE],
                                             pr[:, 1], start=(kc == 0), stop=(kc == KC - 1))
                        for half, av_ps, h in ((0, av_e, he), (64, av_o, ho)):
                            nc.vector.tensor_copy(scr[half:half + 64, dc, :],
                                                  av_ps[0:64, :])
                            tmp = smalls.tile([1, SQ], F32, tag="sumrow")
                            nc.vector.tensor_copy(tmp[:], av_ps[64:65, :])
                            nc.sync.dma_start(sums[h:h + 1, :], tmp[:])
                # reciprocal of denominators; broadcast across each head's 64
                # partitions with a K=16 selector matmul; apply to raw attn@V
                nc.scalar.activation(lnsum[:], sums[:], AF.Ln, bias=eps_t[0:H])
                nc.scalar.activation(recip[:], lnsum[:], AF.Exp, scale=-1.0)
                with tc.tile_pool(name="divps", bufs=2, space="PSUM") as divps:
                    for dc in range(DC):
                        rb_ps = divps.tile([P, SQ], F32, tag="rb")
                        nc.tensor.matmul(rb_ps[:], selr[:, dc * P:(dc + 1) * P],
                                         recip[:], start=True, stop=True)
                        nc.vector.tensor_tensor(out_bf[:, dc], scr[:, dc],
                                                rb_ps[:], ALU.mult)

            def o_proj_residual(av_bf, w_dram, wpool, pp):
                wre = w_dram.rearrange("(ic p) o -> p ic o", p=P)
                for oc in range(DC):
                    wsb = wpool.tile([P, DC, P], BF16, tag="w")
                    nc.sync.dma_start(wsb[:], wre[:, :, oc * P:(oc + 1) * P])
                    pt = pp.tile([P, SQ], F32, tag="pp")
                    for ic in range(DC):
                        nc.tensor.matmul(pt[:], wsb[:, ic], av_bf[:, ic],
                                         start=(ic == 0), stop=(ic == DC - 1))
                    nc.vector.tensor_add(xT[:, oc], xT[:, oc], pt[:])

            # ============================================================ CA ====
            with tc.tile_pool(name="ca", bufs=1) as ca, \
                 tc.tile_pool(name="caw", bufs=4) as caw:
                condT = ca.tile([P, CC, L], BF16)
                nc.sync.dma_start(condT[:],
                                  condT_d.rearrange("(cc p) l -> p cc l", p=P))
                layer_norm(lnp.get("ln1_b"))
                qca = ca.tile([P, DC, SQ], BF16)
                kca = ca.tile([P, DC, L], BF16)
                vca = ca.tile([P, KC_CA, H * HE], BF16)
                nc.gpsimd.memset(
                    vca[:].rearrange("p k (h e) -> p k h e", e=HE)[:, :, :, 64:65],
                    1.0)
                biasT_ca = ca.tile([P, KC_CA, SQ], BF16)
                for kc in range(KC_CA):
                    nc.sync.dma_start_transpose(
                        biasT_ca[:, kc], ca_dist_d[:, kc * P:(kc + 1) * P])

                with tc.tile_pool(name="pca", bufs=2, space="PSUM") as pp:
                    proj_fm(xn, wq_ca_d, DC, DC, caw, pp,
                            lambda oc, pt: nc.vector.tensor_copy(qca[:, oc], pt[:]))
                    # kT: [d, l] = wk.T @ condT; N = L = 1024 -> two 512 halves
                    wkre = wk_ca_d.rearrange("(ic p) o -> p ic o", p=P)
                    for oc in range(DC):
                        wsb = caw.tile([P, CC, P], BF16, tag="w")
                        nc.sync.dma_start(wsb[:], wkre[:, :, oc * P:(oc + 1) * P])
                        for nq in range(2):
                            pt = pp.tile([P, SQ], F32, tag="pp")
                            for ic in range(CC):
                                nc.tensor.matmul(
                                    pt[:], wsb[:, ic],
                                    condT[:, ic, nq * SQ:(nq + 1) * SQ],
                                    start=(ic == 0), stop=(ic == CC - 1))
                            nc.vector.tensor_copy(
                                kca[:, oc, nq * SQ:(nq + 1) * SQ], pt[:])
                    # V (seq-major, strided into vca with ones cols kept)
                    wvca = ca.tile([P, CC, D], BF16)
                    nc.sync.dma_start(wvca[:],
                                      wv_ca_d.rearrange("(ic p) o -> p ic o", p=P))
                    for lc in range(KC_CA):
                        for nd in range(2):
                            pt = pp.tile([P, SQ], F32, tag="pp")
                            for ic in range(CC):
                                nc.tensor.matmul(
                                    pt[:], condT[:, ic, lc * P:(lc + 1) * P],
                                    wvca[:, ic, nd * SQ:(nd + 1) * SQ],
                                    start=(ic == 0), stop=(ic == CC - 1))
                            dst = vca[:, lc].rearrange(
                                "p (h e) -> p h e", e=HE)[:, nd * 8:(nd + 1) * 8, 0:64]
                            nc.vector.tensor_copy(
                                dst, pt[:].rearrange("p (h e) -> p h e", e=64))

                attention(qca, kca, KC_CA, vca, biasT_ca, negg2I["ca"], xn)
                with tc.tile_pool(name="poc", bufs=2, space="PSUM") as pp2:
                    o_proj_residual(xn, wo_ca_d, caw, pp2)

            # ============================================================ SA ====
            layer_norm(lnp.get("ln2_b"))
            k_bounce = dram.tile([P, DC * SQ], BF16)
            k_gath = dram.tile([GROUP, P, DC * SQ], BF16)
            v_bounce = dram.tile([P, 4 * D], BF16)
            v_gath = dram.tile([GROUP, P, 4 * D], BF16)
            with tc.tile_pool(name="sa", bufs=1) as sa, \
                 tc.tile_pool(name="saw", bufs=4) as saw, \
                 tc.tile_pool(name="stg", bufs=1) as stg:
                qsa = sa.tile([P, DC, SQ], BF16)
                kg = sa.tile([P, DC, S], BF16)
                biasT_sa = sa.tile([P, KC_SA, SQ], BF16)
                for kc in range(KC_SA):
                    nc.sync.dma_start_transpose(
                        biasT_sa[:, kc], sa_dist_d[:, kc * P:(kc + 1) * P])
                vext = sa.tile([P, KC_SA, H * HE], BF16)
                nc.gpsimd.memset(
                    vext[:].rearrange("p k (h e) -> p k h e", e=HE)[:, :, :, 64:65],
                    1.0)
                rg = [[0, 1, 2, 3], [4, 5, 6, 7]]
                with tc.tile_pool(name="psa", bufs=2, space="PSUM") as pp:
                    # own K first -> stage -> bounce -> AllGather ASAP
                    kstage = stg.tile([P, DC, SQ], BF16, tag="stage")
                    proj_fm(xn, wk_sa_d, DC, DC, saw, pp,
                            lambda oc, pt: nc.vector.tensor_copy(kstage[:, oc], pt[:]))
                    nc.sync.dma_start(k_bounce[:], kstage[:].rearrange("p a b -> p (a b)"))
                    if with_collective:
                        nc.gpsimd.collective_compute(
                            "AllGather", ALU.bypass,
                            ins=[k_bounce.opt()], outs=[k_gath.opt()],
                            replica_groups=rg)
                    else:
                        for r in range(GROUP):
                            nc.sync.dma_start(k_gath[r], k_bounce[:])
                    # own V (seq-major) -> stage -> bounce -> AllGather
                    wvsa = sa.tile([P, DC, D], BF16)
                    nc.sync.dma_start(wvsa[:],
                                      wv_sa_d.rearrange("(ic p) o -> p ic o", p=P))
                    vstage = stg.tile([P, 4, D], BF16, tag="stage")
                    for sc in range(4):
                        for nd in range(2):
                            pt = pp.tile([P, SQ], F32, tag="pp")
                            for ic in range(DC):
                                nc.tensor.matmul(
                                    pt[:], xn[:, ic, sc * P:(sc + 1) * P],
                                    wvsa[:, ic, nd * SQ:(nd + 1) * SQ],
                                    start=(ic == 0), stop=(ic == DC - 1))
                            nc.vector.tensor_copy(
                                vstage[:, sc, nd * SQ:(nd + 1) * SQ], pt[:])
                    nc.sync.dma_start(v_bounce[:], vstage[:].rearrange("p a b -> p (a b)"))
                    if with_collective:
                        nc.gpsimd.collective_compute(
                            "AllGather", ALU.bypass,
                            ins=[v_bounce.opt()], outs=[v_gath.opt()],
                            replica_groups=rg)
                    else:
                        for r in range(GROUP):
                            nc.sync.dma_start(v_gath[r], v_bounce[:])
                    # Q overlaps the collectives
                    proj_fm(xn, wq_sa_d, DC, DC, saw, pp,
                            lambda oc, pt: nc.vector.tensor_copy(qsa[:, oc], pt[:]))

                for r in range(GROUP):
                    nc.sync.dma_start(
                        kg[:, :, r * SQ:(r + 1) * SQ],
                        k_gath[r].rearrange("p (dc s) -> p dc s", s=SQ))
                    for sc in range(4):
                        src = v_gath[r].rearrange(
                            "p (sc d) -> p sc d", d=D)[:, sc].rearrange(
                            "p (h e) -> p h e", e=64)
                        dst = vext[:, r * 4 + sc].rearrange(
                            "p (h e) -> p h e", e=HE)[:, :, 0:64]
                        nc.sync.dma_start(dst, src)

                attention(qsa, kg, KC_SA, vext, biasT_sa, negg2I["sa"], xn)
                with tc.tile_pool(name="pos", bufs=2, space="PSUM") as pp2:
                    o_proj_residual(xn, wo_sa_d, saw, pp2)

            # =========================================================== MLP ====
            layer_norm(lnp.get("ln3_b"))
            with tc.tile_pool(name="mlp", bufs=1) as mlp, \
                 tc.tile_pool(name="w1p", bufs=4) as w1p, \
                 tc.tile_pool(name="w2p", bufs=3) as w2p:
                h_bf = mlp.tile([P, FC, SQ], BF16)
                w1re = w1_d.rearrange("(ic p) o -> p ic o", p=P)
                w2re = w2_d.rearrange("(f p) o -> p f o", p=P)
                with tc.tile_pool(name="pm1", bufs=2, space="PSUM") as pp:
                    for fc in range(FC):
                        wsb = w1p.tile([P, DC, P], BF16, tag="w1")
                        nc.sync.dma_start(wsb[:], w1re[:, :, fc * P:(fc + 1) * P])
                        pt = pp.tile([P, SQ], F32, tag="pp")
                        for ic in range(DC):
                            nc.tensor.matmul(pt[:], wsb[:, ic], xn[:, ic],
                                             start=(ic == 0), stop=(ic == DC - 1))
                        nc.scalar.activation(h_bf[:, fc], pt[:], AF.Gelu,
                                             bias=b1r[:, fc:fc + 1])
                    for oc in range(DC):
                        wsb = w2p.tile([P, FC, P], BF16, tag="w2")
                        nc.sync.dma_start(wsb[:], w2re[:, :, oc * P:(oc + 1) * P])
                        pt = pp.tile([P, SQ], F32, tag="pp")
                        for fc in range(FC):
                            nc.tensor.matmul(pt[:], wsb[:, fc], h_bf[:, fc],
                                             start=(fc == 0), stop=(fc == FC - 1))
                        nc.vector.tensor_add(scr[:, oc], xT[:, oc], pt[:])
                        nc.vector.tensor_scalar_add(scr[:, oc], scr[:, oc],
                                                    b2r[:, oc:oc + 1])
                        nc.sync.dma_start(out_re[:, oc], scr[:, oc])

    nc.compile()
    return nc


# ---------------------------------------------------------------- host wrapper
_cache = {}
_lock = threading.Lock()


def _get_nc():
    with _lock:
        if "nc" not in _cache:
            _cache["nc"] = build_bass()
        return _cache["nc"]


def _prep_in_maps(x, cond, sa_distance_matrix, ca_distance_matrix,
                  gamma_ca, gamma_sa,
                  ln1_w, ln1_b, ln2_w, ln2_b, ln3_w, ln3_b,
                  ca_wq, ca_wk, ca_wv, ca_wo, sa_wq, sa_wk, sa_wv, sa_wo,
                  mlp_w1, mlp_b1, mlp_w2, mlp_b2):
    bf = lambda a: np.ascontiguousarray(a).astype(BF16NP)
    f32 = lambda a: np.ascontiguousarray(a, dtype=np.float32)
    scale = 1.0 / np.sqrt(HD)
    w1, w2, w3 = (f32(ln1_w)[:, None], f32(ln2_w)[:, None], f32(ln3_w)[:, None])

    # ln weights fold into the next projections' input dim (wT rows)
    shared = dict(
        g_sa=f32(gamma_sa).reshape(1, 1), g_ca=f32(gamma_ca).reshape(1, 1),
        wq_ca=bf((ca_wq * scale).T * w1), wk_ca=bf(ca_wk.T), wv_ca=bf(ca_wv.T),
        wo_ca=bf(ca_wo.T),
        wq_sa=bf((sa_wq * scale).T * w2), wk_sa=bf(sa_wk.T * w2),
        wv_sa=bf(sa_wv.T * w2), wo_sa=bf(sa_wo.T),
        w1T=bf(mlp_w1.T * w3), w2T=bf(mlp_w2.T),
        b1r=f32(mlp_b1).reshape(FC, P).T.copy(),
        b2r=f32(mlp_b2).reshape(DC, P).T.copy(),
    )
    sel = np.zeros((H, D), np.float32)
    for h in range(H):
        sel[h, (h // 2) * P + (h % 2) * HD:(h // 2) * P + (h % 2) * HD + HD] = 1.0
    shared["selr"] = bf(sel)
    if any(np.any(np.asarray(b) != 0) for b in (ln1_b, ln2_b, ln3_b)):
        raise NotImplementedError(
            "nonzero ln bias: rebuild with build_bass(apply_lnb=True) and pass "
            "ln{1,2,3}_b as [P, DC] inputs")

    in_maps = []
    for core in range(NCORES):
        b, r = core // GROUP, core % GROUP
        q0 = r * SQ
        m = dict(shared)
        m["xT"] = f32(x[b, q0:q0 + SQ, :].T)
        m["condT"] = bf(cond[b].T)
        m["sa_dist"] = bf(sa_distance_matrix[b, q0:q0 + SQ, :])
        m["ca_dist"] = bf(ca_distance_matrix[b, q0:q0 + SQ, :])
        in_maps.append(m)
    return in_maps


def kernel(**inputs):
    from concourse.bass_utils import run_bass_kernel_spmd

    nc = _get_nc()
    in_maps = _prep_in_maps(**inputs)
    res = run_bass_kernel_spmd(nc, in_maps, core_ids=list(range(NCORES)))
    out = np.empty((B, S, D), np.float32)
    for core in range(NCORES):
        b, r = core // GROUP, core % GROUP
        out[b, r * SQ:(r + 1) * SQ, :] = res.results[core]["outT"].T
    return out
